# revision 1
# baseline (speedup 1.0000x reference)
"""Multi-head attention (B=4, L=2048, D=1024, H=16) on 8 NeuronCores.

Sharding: core c handles batch b=c//2 and query rows [1024*(c%2), +1024).
The per-core input x is the batch's [2048, 1024] activations ROTATED so the
core's own query rows are rows 0..1023 (softmax over keys is permutation
invariant, so rotating keys+values together is exact). No collectives needed.

Per-core pipeline (all matmuls in float32r = full-speed ~tf32 precision):
  A)  transpose x -> XT [k, s] (PE transpose); QT = Wq^T@XT[:, :1024],
      KT = Wk^T@XT (SBUF resident); V = XT^T@Wv staged to DRAM with a
      fused ones-column per head (for the softmax denominator).
  B1) per head pair: scores^T tile [s,l] = KT_h^T @ QT_h (contraction d=64,
      row-group paired across the 2 heads); exp via ScalarE (scale=1/8
      folded); PV accumulate [V_h|1]^T @ exp(S^T) -> [65, l] PSUM where row
      64 = softmax denominator; normalize rows 0..63 by broadcasted
      reciprocal.
  C)  y^T = Wo^T @ OT (+bo fused), PE-transpose back to [l, dout], DMA out.
"""

import numpy as np

import sys

for _p in ("/opt/trn_rl_repo", "/opt/pypackages"):
    if _p not in sys.path:
        sys.path.append(_p)

from contextlib import ExitStack

import concourse.bass as bass
import concourse.mybir as mybir
import concourse.tile as tile
from concourse import bacc
from concourse.bass_utils import run_bass_kernel_spmd
from concourse.masks import make_identity

B, L, D, H = 4, 2048, 1024, 16
HD = D // H  # 64
LQ = 1024  # query rows per core
N_CORES = 8
F32 = mybir.dt.float32
F32R = mybir.dt.float32r
AF = mybir.ActivationFunctionType

P = 128
KT_TILES = D // P  # 8 k tiles
ST_TILES = L // P  # 16 s tiles
DT_TILES = D // P  # 8 d tiles
LH = 512  # l half width
SCALE = 1.0 / float(np.sqrt(HD))
PIPELINE = True
COMBINED_EXP = True
B1_LHALF = True
B1_XPAIR = True


def _load_bias(nc, pool, dram, name):
    """[1024] dram vector -> [128, 8] sbuf tile; column t = b[128t:128t+128]."""
    t = pool.tile([P, DT_TILES], F32, name=name)
    nc.gpsimd.dma_start(t[:], dram.rearrange("(t p) -> p t", p=P))
    return t


def build_nc(repeat=1, stop_after=None):
    nc = bacc.Bacc(None)

    x_d = nc.declare_dram_parameter("x", [L, D], F32, isOutput=False)
    wq_d = nc.declare_dram_parameter("wq", [D, D], F32, isOutput=False)
    wk_d = nc.declare_dram_parameter("wk", [D, D], F32, isOutput=False)
    wv_d = nc.declare_dram_parameter("wv", [D, D], F32, isOutput=False)
    wo_d = nc.declare_dram_parameter("wo", [D, D], F32, isOutput=False)
    bq_d = nc.declare_dram_parameter("bq", [D], F32, isOutput=False)
    bk_d = nc.declare_dram_parameter("bk", [D], F32, isOutput=False)
    bv_d = nc.declare_dram_parameter("bv", [D], F32, isOutput=False)
    bo_d = nc.declare_dram_parameter("bo", [D], F32, isOutput=False)
    y_d = nc.declare_dram_parameter("y", [LQ, D], F32, isOutput=True)

    # V staged in DRAM, already augmented with a ones column per head:
    # [s_tile, partition(s), head, 65] where col 64 of each head slot is 1.0
    v_dram = nc.dram_tensor("v_stage", [ST_TILES, P, H, HD + 1], F32R)

    with tile.TileContext(nc) as tc, ExitStack() as ctx:
      for _rep in range(repeat):
       with ExitStack() as rctx:
        singles = rctx.enter_context(tc.tile_pool(name="singles", bufs=1))
        ident32 = singles.tile([P, P], F32, name="ident32")
        make_identity(nc, ident32[:])
        ident = singles.tile([P, P], F32R, name="ident")
        nc.vector.tensor_copy(ident[:], ident32[:])
        bq_sb = _load_bias(nc, singles, bq_d, "bq")
        bk_sb = _load_bias(nc, singles, bk_d, "bk")
        bv_sb = _load_bias(nc, singles, bv_d, "bv")
        bo_sb = _load_bias(nc, singles, bo_d, "bo")

        # big resident slabs
        qt_pool = rctx.enter_context(tc.tile_pool(name="qt", bufs=1))
        kt_pool = rctx.enter_context(tc.tile_pool(name="kt", bufs=1))
        qt = qt_pool.tile([P, DT_TILES, LQ], F32R, name="qt")  # [d%128, dtile, l]
        kt = kt_pool.tile([P, DT_TILES, L], F32R, name="kt")  # [d%128, dtile, s]

        # ---------------- Phase A: transpose + projections ----------------
        with (
            tc.tile_pool(name="xt", bufs=1) as xt_pool,
            tc.tile_pool(name="wpool", bufs=2) as wpool,
            tc.tile_pool(name="vb", bufs=3) as vb_pool,
            tc.tile_pool(name="wv", bufs=1) as wv_pool,
            tc.tile_pool(name="ps_proj", bufs=4, space="PSUM") as ps_proj,
        ):
            xt = xt_pool.tile([P, KT_TILES, L], F32R, name="xt")  # [k%128, ktile, s]

            # transpose x into xt (PE transpose of 128x128 blocks)
            with (
                tc.tile_pool(name="xpool", bufs=3) as xpool,
                tc.tile_pool(name="ps_tr", bufs=3, space="PSUM") as ps_tr,
            ):
                for li in range(ST_TILES):
                    # plain HWDGE fp32 load; fp32->fp32r cast happens for free
                    # in the transpose-evict copy below (4 transposes batched
                    # into one PSUM bank -> single DVE eviction)
                    x_sb = xpool.tile([P, D], F32, name="x_sb")
                    nc.sync.dma_start(x_sb[:], x_d[li * P : (li + 1) * P, :])
                    for kg in range(KT_TILES // 4):
                        pt4 = ps_tr.tile([P, 4, P], F32, name="pt4")
                        for b in range(4):
                            ki = 4 * kg + b
                            nc.tensor.transpose(
                                pt4[:, b, :], x_sb[:, ki * P : (ki + 1) * P], ident32[:]
                            )
                        nc.vector.tensor_copy(
                            xt[:, 4 * kg : 4 * kg + 4, li * P : (li + 1) * P], pt4[:]
                        )

            # QT[d, l] = sum_k Wq[k, d-tile]^T @ XT[k, l]   (+bq fused)
            # KT[d, s] = sum_k Wk[k, d-tile]^T @ XT[k, s]   (+bk fused)
            # W column block per d-tile: [128(k%128), ktile, 128(d)]
            for w_d, b_sb, out_sb, ncols in (
                (wq_d, bq_sb, qt, LQ),
                (wk_d, bk_sb, kt, L),
            ):
                for dt_i in range(DT_TILES):
                    w_col = wpool.tile([P, KT_TILES, P], F32R, name="w_col")
                    nc.gpsimd.dma_start(
                        w_col[:],
                        w_d[:, dt_i * P : (dt_i + 1) * P].rearrange(
                            "(t p) n -> p t n", p=P
                        ),
                    )
                    for ci in range(ncols // LH):
                        ps = ps_proj.tile([P, LH], F32, name="ps_proj")
                        for ki in range(KT_TILES):
                            nc.tensor.matmul(
                                ps[:],
                                w_col[:, ki, :],
                                xt[:, ki, ci * LH : (ci + 1) * LH],
                                start=(ki == 0),
                                stop=(ki == KT_TILES - 1),
                            )
                        nc.scalar.activation(
                            out_sb[:, dt_i, ci * LH : (ci + 1) * LH],
                            ps[:],
                            AF.Identity,
                            bias=b_sb[:, dt_i : dt_i + 1],
                        )

            # V[s, d] = sum_k XT[k, s-tile]^T @ Wv[k, d] staged to DRAM
            # bounce buffer interleaves the per-head ones column.
            for dc in range(2):  # 512-wide chunks = 8 heads each
                wv_half = wv_pool.tile([P, KT_TILES, LH], F32R, name="wv_half")
                nc.gpsimd.dma_start(
                    wv_half[:],
                    wv_d[:, dc * LH : (dc + 1) * LH].rearrange("(t p) n -> p t n", p=P),
                )
                for st in range(ST_TILES):
                    ps = ps_proj.tile([P, LH], F32, name="ps_proj")
                    for ki in range(KT_TILES):
                        nc.tensor.matmul(
                            ps[:],
                            xt[:, ki, st * P : (st + 1) * P],
                            wv_half[:, ki, :],
                            start=(ki == 0),
                            stop=(ki == KT_TILES - 1),
                        )
                    vb = vb_pool.tile([P, 8, HD + 1], F32R, name="vb")
                    nc.vector.memset(vb[:, :, HD : HD + 1].bitcast(F32), 1.0)
                    nc.vector.tensor_copy(vb[:, :, 0:HD], ps[:])
                    nc.sync.dma_start(v_dram[st, :, dc * 8 : (dc + 1) * 8, :], vb[:])

        if stop_after == "a":
            for i in range(KT_TILES):
                nc.sync.dma_start(y_d[i * P : (i + 1) * P, :], qt[:, i, :].bitcast(F32))
            continue

        # ---------------- Phase B1: attention per head pair ----------------
        ot_pool = rctx.enter_context(tc.tile_pool(name="ot", bufs=1))
        ot = ot_pool.tile([P, DT_TILES, LQ], F32R, name="ot")  # [din%128, dintile, l]

        with (
            tc.tile_pool(name="vaug", bufs=2) as vaug_pool,
            tc.tile_pool(name="et", bufs=(5 if B1_LHALF else 3 if COMBINED_EXP else 6)) as et_pool,
            tc.tile_pool(name="otmp", bufs=3) as otmp_pool,
            tc.tile_pool(name="rr", bufs=2) as rr_pool,
            tc.tile_pool(name="rb", bufs=2) as rb_pool,
            tc.tile_pool(name="ps_s", bufs=(2 if B1_LHALF else 1 if COMBINED_EXP else 2), space="PSUM") as ps_s_pool,
            tc.tile_pool(name="ps_o", bufs=2, space="PSUM") as ps_o_pool,
        ):
            if B1_XPAIR:
                # flat unit pipeline across pair boundaries: the lookahead-1
                # scores/exp never drains at a pair boundary
                pair_vaug = {}
                pair_pso = {}

                def ensure_vaug(p):
                    if p not in pair_vaug:
                        v = vaug_pool.tile(
                            [P, ST_TILES, 2 * (HD + 1)], F32R, name="vaug"
                        )
                        nc.sync.dma_start(
                            v[:],
                            v_dram[:, :, 2 * p : 2 * p + 2, :].rearrange(
                                "s p h c -> p s (h c)"
                            ),
                        )
                        pair_vaug[p] = v

                def scores_g(p, st, lh):
                    ps_s = ps_s_pool.tile([P, 2, LH], F32, name="ps_s")
                    for sub in range(2):
                        nc.tensor.matmul(
                            ps_s[:, sub, :],
                            kt[sub * HD : (sub + 1) * HD, p, st * P : (st + 1) * P],
                            qt[sub * HD : (sub + 1) * HD, p, lh * LH : (lh + 1) * LH],
                            start=True,
                            stop=True,
                        )
                    e2 = et_pool.tile([P, 2, LH], F32R, name="et")
                    nc.scalar.activation(e2[:], ps_s[:], AF.Exp, scale=SCALE)
                    return e2

                def pv_g(p, st, lh, e2):
                    if p not in pair_pso:
                        pair_pso[p] = [
                            ps_o_pool.tile([HD + 1, LQ], F32, name="ps_o")
                            for _ in range(2)
                        ]
                    po = pair_pso[p]
                    for sub in range(2):
                        nc.tensor.matmul(
                            po[sub][:, lh * LH : (lh + 1) * LH],
                            pair_vaug[p][:, st, sub * (HD + 1) : (sub + 1) * (HD + 1)],
                            e2[:, sub, :],
                            start=(st == 0),
                            stop=(st == ST_TILES - 1),
                        )

                def epilogue(p):
                    po = pair_pso.pop(p)
                    pair_vaug.pop(p)
                    for sub in range(2):
                        o_tmp = otmp_pool.tile([HD + 1, LQ], F32, name="o_tmp")
                        nc.vector.tensor_copy(o_tmp[:], po[sub][:])
                        r_row = rr_pool.tile([1, LQ], F32, name="r_row")
                        nc.vector.reciprocal(r_row[:], o_tmp[HD : HD + 1, :])
                        r_bc = rb_pool.tile([HD, LQ], F32, name="r_bc")
                        nc.gpsimd.partition_broadcast(r_bc[:], r_row[:])
                        dst = ot[sub * HD : (sub + 1) * HD, p, :]
                        nc.vector.tensor_mul(dst, o_tmp[0:HD, :], r_bc[:])
                        nc.vector.tensor_scalar_add(
                            dst, dst, bv_sb[sub * HD : (sub + 1) * HD, p : p + 1]
                        )

                all_units = [
                    (p, st, lh)
                    for p in range(H // 2)
                    for st in range(ST_TILES)
                    for lh in range(2)
                ]
                prev = None
                for u in all_units:
                    ensure_vaug(u[0])
                    e2 = scores_g(*u)
                    if prev is not None:
                        pv_g(*prev[0], prev[1])
                        if prev[0][1] == ST_TILES - 1 and prev[0][2] == 1:
                            epilogue(prev[0][0])
                    prev = (u, e2)
                pv_g(*prev[0], prev[1])
                epilogue(prev[0][0])

            for pair in ([] if B1_XPAIR else range(H // 2)):
                vaug = vaug_pool.tile([P, ST_TILES, 2 * (HD + 1)], F32R, name="vaug")
                nc.sync.dma_start(
                    vaug[:],
                    v_dram[:, :, 2 * pair : 2 * pair + 2, :].rearrange(
                        "s p h c -> p s (h c)"
                    ),
                )
                ps_o = [
                    ps_o_pool.tile([HD + 1, LQ], F32, name="ps_o") for _ in range(2)
                ]

                def scores_exp_lh(st, lh):
                    # 2-bank scores tile (both subs, one l-half): restores
                    # ps_s double-buffering within the 8-bank PSUM budget
                    ps_s = ps_s_pool.tile([P, 2, LH], F32, name="ps_s")
                    for sub in range(2):
                        nc.tensor.matmul(
                            ps_s[:, sub, :],
                            kt[sub * HD : (sub + 1) * HD, pair, st * P : (st + 1) * P],
                            qt[sub * HD : (sub + 1) * HD, pair, lh * LH : (lh + 1) * LH],
                            start=True,
                            stop=True,
                        )
                    e2 = et_pool.tile([P, 2, LH], F32R, name="et")
                    nc.scalar.activation(e2[:], ps_s[:], AF.Exp, scale=SCALE)
                    return e2

                def pv_lh(st, lh, e2):
                    for sub in range(2):
                        nc.tensor.matmul(
                            ps_o[sub][:, lh * LH : (lh + 1) * LH],
                            vaug[:, st, sub * (HD + 1) : (sub + 1) * (HD + 1)],
                            e2[:, sub, :],
                            start=(st == 0),
                            stop=(st == ST_TILES - 1),
                        )

                def scores_exp(st):
                    if COMBINED_EXP:
                        # both heads' scores into one 4-bank PSUM tile so a
                        # SINGLE [128, 2048] ACTIVATE covers them (halves the
                        # per-op ScalarE overhead)
                        ps_s = ps_s_pool.tile([P, 2, LQ], F32, name="ps_s")
                        for sub in range(2):
                            for lh in range(2):
                                nc.tensor.matmul(
                                    ps_s[:, sub, lh * LH : (lh + 1) * LH],
                                    kt[sub * HD : (sub + 1) * HD, pair, st * P : (st + 1) * P],
                                    qt[sub * HD : (sub + 1) * HD, pair, lh * LH : (lh + 1) * LH],
                                    start=True,
                                    stop=True,
                                )
                        e2 = et_pool.tile([P, 2, LQ], F32R, name="et")
                        nc.scalar.activation(e2[:], ps_s[:], AF.Exp, scale=SCALE)
                        return [e2[:, 0, :], e2[:, 1, :]]
                    et = [None, None]
                    for sub in range(2):
                        ps_s = ps_s_pool.tile([P, LQ], F32, name="ps_s")
                        for lh in range(2):
                            nc.tensor.matmul(
                                ps_s[:, lh * LH : (lh + 1) * LH],
                                kt[sub * HD : (sub + 1) * HD, pair, st * P : (st + 1) * P],
                                qt[sub * HD : (sub + 1) * HD, pair, lh * LH : (lh + 1) * LH],
                                start=True,
                                stop=True,
                            )
                        e = et_pool.tile([P, LQ], F32R, name="et")
                        nc.scalar.activation(e[:], ps_s[:], AF.Exp, scale=SCALE)
                        et[sub] = e
                    return et

                def pv(st, et):
                    for sub in range(2):
                        for lh in range(2):
                            nc.tensor.matmul(
                                ps_o[sub][:, lh * LH : (lh + 1) * LH],
                                vaug[:, st, sub * (HD + 1) : (sub + 1) * (HD + 1)],
                                et[sub][:, lh * LH : (lh + 1) * LH],
                                start=(st == 0),
                                stop=(st == ST_TILES - 1),
                            )

                if B1_LHALF:
                    units = [(st, lh) for st in range(ST_TILES) for lh in range(2)]
                    e_cur = scores_exp_lh(*units[0])
                    for i, u in enumerate(units):
                        e_next = (
                            scores_exp_lh(*units[i + 1]) if i + 1 < len(units) else None
                        )
                        pv_lh(*u, e_cur)
                        e_cur = e_next
                elif PIPELINE:
                    # software pipeline: scores(st+1) emitted before pv(st) so
                    # PE has independent work while ACT computes exp(st)
                    et_cur = scores_exp(0)
                    for st in range(ST_TILES):
                        et_next = scores_exp(st + 1) if st + 1 < ST_TILES else None
                        pv(st, et_cur)
                        et_cur = et_next
                else:
                    for st in range(ST_TILES):
                        pv(st, scores_exp(st))
                # evict O+denominator to SBUF immediately (frees the PSUM
                # bank for the next pair), then normalize rows 0..63 by the
                # broadcasted reciprocal of row 64, write into ot slab (+bv).
                for sub in range(2):
                    o_tmp = otmp_pool.tile([HD + 1, LQ], F32, name="o_tmp")
                    nc.vector.tensor_copy(o_tmp[:], ps_o[sub][:])
                    r_row = rr_pool.tile([1, LQ], F32, name="r_row")
                    nc.vector.reciprocal(r_row[:], o_tmp[HD : HD + 1, :])
                    r_bc = rb_pool.tile([HD, LQ], F32, name="r_bc")
                    nc.gpsimd.partition_broadcast(r_bc[:], r_row[:])
                    dst = ot[sub * HD : (sub + 1) * HD, pair, :]
                    nc.vector.tensor_mul(dst, o_tmp[0:HD, :], r_bc[:])
                    nc.vector.tensor_scalar_add(
                        dst, dst, bv_sb[sub * HD : (sub + 1) * HD, pair : pair + 1]
                    )

        if stop_after == "ab":
            for i in range(KT_TILES):
                nc.sync.dma_start(y_d[i * P : (i + 1) * P, :], ot[:, i, :].bitcast(F32))
            continue

        # ---------------- Phase C: output projection + transpose ----------------
        with (
            tc.tile_pool(name="wo", bufs=2) as wo_pool,
            tc.tile_pool(name="gt", bufs=2) as gt_pool,
            tc.tile_pool(name="ysl", bufs=1) as y_pool,
            tc.tile_pool(name="ps_g", bufs=2, space="PSUM") as ps_g_pool,
            tc.tile_pool(name="ps_t", bufs=3, space="PSUM") as ps_t_pool,
        ):
            y_sb = y_pool.tile([P, KT_TILES, D], F32, name="y_sb")  # [l%128, ltile, dout]
            for j in range(DT_TILES):  # dout tiles
                wo_sb = wo_pool.tile([P, KT_TILES, P], F32R, name="wo_sb")
                nc.gpsimd.dma_start(
                    wo_sb[:],
                    wo_d[:, j * P : (j + 1) * P].rearrange("(t p) n -> p t n", p=P),
                )
                gt_s = gt_pool.tile([P, LQ], F32R, name="gt_s")
                for lh in range(2):
                    ps_g = ps_g_pool.tile([P, LH], F32, name="ps_g")
                    for ki in range(KT_TILES):
                        nc.tensor.matmul(
                            ps_g[:],
                            wo_sb[:, ki, :],
                            ot[:, ki, lh * LH : (lh + 1) * LH],
                            start=(ki == 0),
                            stop=(ki == KT_TILES - 1),
                        )
                    nc.scalar.activation(
                        gt_s[:, lh * LH : (lh + 1) * LH],
                        ps_g[:],
                        AF.Identity,
                        bias=bo_sb[:, j : j + 1],
                    )
                for a in range(KT_TILES // 4):  # l tiles, batched 4-per-bank
                    pt4 = ps_t_pool.tile([P, 4, P], F32R, name="pt4_out")
                    for b in range(4):
                        i = 4 * a + b
                        nc.tensor.transpose(
                            pt4[:, b, :], gt_s[:, i * P : (i + 1) * P], ident[:]
                        )
                    nc.vector.tensor_copy(
                        y_sb[:, 4 * a : 4 * a + 4, j * P : (j + 1) * P], pt4[:]
                    )
            for i in range(KT_TILES):
                nc.sync.dma_start(y_d[i * P : (i + 1) * P, :], y_sb[:, i, :])

    nc.finalize()
    return nc


_NC_CACHE = None


def kernel(**inputs):
    global _NC_CACHE
    if _NC_CACHE is None:
        _NC_CACHE = build_nc()
    nc = _NC_CACHE

    q = np.ascontiguousarray(np.asarray(inputs["q"], dtype=np.float32))
    w = {k: np.ascontiguousarray(np.asarray(inputs[k], dtype=np.float32))
         for k in ("Wq", "Wk", "Wv", "Wo", "bq", "bk", "bv", "bo")}

    in_maps = []
    for c in range(N_CORES):
        b, half = c // 2, c % 2
        lo = LQ * half
        x_rot = np.concatenate([q[b, lo:], q[b, :lo]], axis=0)
        in_maps.append({
            "x": np.ascontiguousarray(x_rot),
            "wq": w["Wq"], "wk": w["Wk"], "wv": w["Wv"], "wo": w["Wo"],
            "bq": w["bq"], "bk": w["bk"], "bv": w["bv"], "bo": w["bo"],
        })

    res = run_bass_kernel_spmd(nc, in_maps, core_ids=list(range(N_CORES)))

    out = np.empty((B, L, D), dtype=np.float32)
    for c in range(N_CORES):
        b, half = c // 2, c % 2
        lo = LQ * half
        out[b, lo : lo + LQ, :] = res.results[c]["y"]
    return out



# revision 9
# speedup vs baseline: 1.0088x; 1.0088x over previous
"""Multi-head attention (B=4, L=2048, D=1024, H=16) on 8 NeuronCores.

Sharding: core c handles batch b=c//2 and query rows [1024*(c%2), +1024).
Per-core input x is the batch's [2048, 1024] activations ROTATED so the
core's own query rows are rows 0..1023 (softmax over keys is permutation
invariant). No collectives.

v2 design (vs v1 baseline):
- ACT (ScalarE) does ONLY exp (the hard 263us/core floor at 1 elem/lane/cyc);
  every PSUM eviction moved to DVE (with bias via tensor_scalar_add or a
  broadcast bo row). Single act table load.
- bf16 residents: xt/qt/kt/V/e2/ot (moving-operand bf16 = 1 cyc/row on PE,
  same as f32r at >=256 cols, but halves SBUF so V stays resident in SBUF --
  no DRAM roundtrip for V).
- Scores matmuls (contraction=64) use PE row tiling: sub0 on partitions 0-63
  -> tile (0,0), sub1 on 64-127 -> tile (64,0); the two stream CONCURRENTLY
  (microbenched ~1.8x), halving scores PE time.
- x transposed with a bf16 identity (1 cyc/row vs 2 for fp32) into bf16 xt.
- Output projection computes y[l,:] directly (lhsT=ot l-block, rhs=Wo) --
  no output transpose pass at all.
- Flat (pair, lh, st) unit pipeline with lookahead-1 exp; QT/KT/V projection
  chains for later head-pairs are interleaved into the attention phase as PE
  "fill" work, deadline-scheduled, so PE mops up its surplus while ACT
  streams exp at 100% duty.

PSUM: ps_s (scores) 2 tiles x 2 banks + po (PV accum, [65,2,512], ones-row
denominator trick) 1 tile x 2 banks + fills 2 x 1 bank = 8 banks exactly.
"""

import numpy as np

import sys

for _p in ("/opt/trn_rl_repo", "/opt/pypackages"):
    if _p not in sys.path:
        sys.path.append(_p)

from contextlib import ExitStack

import concourse.bass as bass
import concourse.mybir as mybir
import concourse.tile as tile
from concourse import bacc
from concourse.bass_utils import run_bass_kernel_spmd
from concourse.masks import make_identity

B, L, D, H = 4, 2048, 1024, 16
HD = D // H  # 64
LQ = 1024  # query rows per core
N_CORES = 8
F32 = mybir.dt.float32
F32R = mybir.dt.float32r
BF16 = mybir.dt.bfloat16
AF = mybir.ActivationFunctionType

P = 128
DT = D // P  # 8 d tiles
ST = L // P  # 16 s tiles
LH = 512
NPAIR = H // 2  # 8 head pairs
SCALE = 1.0 / float(np.sqrt(HD))

# fill chain kinds
QT_CH, KT_CH, V_CH = 0, 1, 2


def _load_bias(nc, pool, dram, name):
    """[1024] dram vector -> [128, 8] sbuf tile; column t = b[128t:128t+128]."""
    t = pool.tile([P, DT], F32, name=name)
    nc.gpsimd.dma_start(t[:], dram.rearrange("(t p) -> p t", p=P))
    return t


def build_nc(repeat=1, stop_after=None):
    nc = bacc.Bacc(None)

    x_d = nc.declare_dram_parameter("x", [L, D], F32, isOutput=False)
    wq_d = nc.declare_dram_parameter("wq", [D, D], F32, isOutput=False)
    wk_d = nc.declare_dram_parameter("wk", [D, D], F32, isOutput=False)
    wv_d = nc.declare_dram_parameter("wv", [D, D], F32, isOutput=False)
    wo_d = nc.declare_dram_parameter("wo", [D, D], F32, isOutput=False)
    bq_d = nc.declare_dram_parameter("bq", [D], F32, isOutput=False)
    bk_d = nc.declare_dram_parameter("bk", [D], F32, isOutput=False)
    bv_d = nc.declare_dram_parameter("bv", [D], F32, isOutput=False)
    bo_d = nc.declare_dram_parameter("bo", [D], F32, isOutput=False)
    y_d = nc.declare_dram_parameter("y", [LQ, D], F32, isOutput=True)

    with tile.TileContext(nc) as tc, ExitStack() as ctx:
      for _rep in range(repeat):
       with ExitStack() as rctx:
        singles = rctx.enter_context(tc.tile_pool(name="singles", bufs=1))
        ident32 = singles.tile([P, P], F32, name="ident32")
        make_identity(nc, ident32[:])
        ident_b = singles.tile([P, P], BF16, name="ident_b")
        nc.vector.tensor_copy(ident_b[:], ident32[:])
        bq_sb = _load_bias(nc, singles, bq_d, "bq")
        bk_sb = _load_bias(nc, singles, bk_d, "bk")
        bv_sb = _load_bias(nc, singles, bv_d, "bv")
        bo_row = singles.tile([1, D], F32, name="bo_row")
        nc.gpsimd.dma_start(bo_row[:], bo_d.rearrange("(a d) -> a d", a=1))
        bo_bc = singles.tile([P, D], F32, name="bo_bc")
        nc.gpsimd.partition_broadcast(bo_bc[:], bo_row[:])

        # big resident slabs (bf16)
        slab = rctx.enter_context(tc.tile_pool(name="slab", bufs=1))
        qt = slab.tile([P, DT, LQ], BF16, name="qt")  # [d%128, dtile, l]
        kt = slab.tile([P, DT, L], BF16, name="kt")  # [d%128, dtile, s]
        ot = slab.tile([P, DT, LQ], BF16, name="ot")  # [din%128, dintile, l]
        # V resident in SBUF: [s%128, st, head, 65]; col 64 = 1.0 (denominator)
        vsb = slab.tile([P, ST, H, HD + 1], BF16, name="vsb")
        nc.vector.memset(vsb[:, :, :, HD : HD + 1], 1.0)

        # wv stays loaded for the V fill chains (rhs moving operand); bf16
        # (gpsimd DMA casts) to fit the SBUF budget
        wv_sb = slab.tile([P, DT, D], BF16, name="wv_sb")
        nc.gpsimd.dma_start(wv_sb[:], wv_d.rearrange("(t p) n -> p t n", p=P))

        with ExitStack() as bctx:
            xt_pool = bctx.enter_context(tc.tile_pool(name="xt", bufs=1))
            xt = xt_pool.tile([P, DT, L], BF16, name="xt")  # [k%128, ktile, s]

            # rotating weight-column tiles for QT/KT fill chains
            wf_pool = bctx.enter_context(tc.tile_pool(name="wf", bufs=4))

            def load_wcol(w_d, dt_i, name):
                w_col = wf_pool.tile([P, DT, P], BF16, name="w_col")
                nc.gpsimd.dma_start(
                    w_col[:],
                    w_d[:, dt_i * P : (dt_i + 1) * P].rearrange(
                        "(t p) n -> p t n", p=P
                    ),
                )
                return w_col

            # ---------------- prefix: transpose + first projections --------
            with (
                tc.tile_pool(name="xpool", bufs=3) as xpool,
                tc.tile_pool(name="ps_tr", bufs=3, space="PSUM") as ps_tr,
                tc.tile_pool(name="ps_pf", bufs=3, space="PSUM") as ps_pf,
            ):
                # transpose x into xt: bf16 identity streams at 1 cyc/row;
                # stationary data bitcast to f32r (bf16-mode truncation is
                # fine -- xt is stored bf16 anyway)
                for li in range(ST):
                    x_sb = xpool.tile([P, D], BF16, name="x_sb")
                    nc.gpsimd.dma_start(x_sb[:], x_d[li * P : (li + 1) * P, :])
                    for kg in range(DT // 4):
                        pt4 = ps_tr.tile([P, 4, P], BF16, name="pt4")
                        for b in range(4):
                            ki = 4 * kg + b
                            nc.tensor.transpose(
                                pt4[:, b, :],
                                x_sb[:, ki * P : (ki + 1) * P],
                                ident_b[:],
                            )
                        nc.vector.tensor_copy(
                            xt[:, 4 * kg : 4 * kg + 4, li * P : (li + 1) * P], pt4[:]
                        )

                def qt_chain(w_col, dt_i, ci, pool):
                    ps = pool.tile([P, LH], F32, name="ps_ch")
                    for ki in range(DT):
                        nc.tensor.matmul(
                            ps[:],
                            w_col[:, ki, :],
                            xt[:, ki, ci * LH : (ci + 1) * LH],
                            start=(ki == 0),
                            stop=(ki == DT - 1),
                        )
                    nc.vector.tensor_scalar_add(
                        qt[:, dt_i, ci * LH : (ci + 1) * LH],
                        ps[:],
                        bq_sb[:, dt_i : dt_i + 1],
                    )

                def kt_chain(w_col, dt_i, ci, pool):
                    ps = pool.tile([P, LH], F32, name="ps_ch")
                    for ki in range(DT):
                        nc.tensor.matmul(
                            ps[:],
                            w_col[:, ki, :],
                            xt[:, ki, ci * LH : (ci + 1) * LH],
                            start=(ki == 0),
                            stop=(ki == DT - 1),
                        )
                    nc.vector.tensor_scalar_add(
                        kt[:, dt_i, ci * LH : (ci + 1) * LH],
                        ps[:],
                        bk_sb[:, dt_i : dt_i + 1],
                    )

                def v_chain(g, st, pool):
                    # V quarter-group g (4 heads = 256 d-cols) for s-tile st
                    ps = pool.tile([P, LH], F32, name="ps_ch")
                    for ki in range(DT):
                        nc.tensor.matmul(
                            ps[:, 0:256],
                            xt[:, ki, st * P : (st + 1) * P],
                            wv_sb[:, ki, g * 256 : (g + 1) * 256],
                            start=(ki == 0),
                            stop=(ki == DT - 1),
                        )
                    # scatter 4 heads' 64-wide blocks into vsb (no bias: bv
                    # is added after normalization in the epilogue)
                    nc.vector.tensor_copy(
                        vsb[:, st, 4 * g : 4 * g + 4, 0:HD],
                        ps[:, 0:256],
                    )

                # prefix chains: QT/KT d-tile 0 and all of V group 0
                wq0 = load_wcol(wq_d, 0, "wq0")
                wk0 = load_wcol(wk_d, 0, "wk0")
                for ci in range(2):
                    qt_chain(wq0, 0, ci, ps_pf)
                for ci in range(4):
                    kt_chain(wk0, 0, ci, ps_pf)
                for st in range(ST):
                    v_chain(0, st, ps_pf)

            if stop_after == "a":
                # dump qt (bf16) widened via DVE into a f32 bounce
                with tc.tile_pool(name="dmp", bufs=2) as dmp:
                    for i in range(DT):
                        t = dmp.tile([P, LQ], F32, name="dmp_t")
                        nc.vector.tensor_copy(t[:], qt[:, i, :])
                        nc.sync.dma_start(y_d[i * P : (i + 1) * P, :], t[:])
                continue

            # ---------------- attention phase with interleaved fills -------
            # fill chains and their deadlines (unit index by which they must
            # be DONE; pair p starts at unit 32p)
            fills = []
            for dt_i in range(1, DT):
                for ci in range(2):
                    fills.append((32 * dt_i, QT_CH, dt_i, ci, 1707))
                for ci in range(4):
                    fills.append((32 * dt_i, KT_CH, dt_i, ci, 1707))
            for g in range(1, 4):
                for st in range(ST):
                    # needed at PV of pair 2g, lh0, st -> unit 64g + st
                    fills.append((64 * g + st, V_CH, g, st, 853))
            fills.sort(key=lambda f: f[0])

            # prefetch weight tiles a couple of d-tiles ahead of use
            wcols = {}

            def ensure_wcol(kind, dt_i):
                key = (kind, dt_i)
                if key not in wcols:
                    wcols[key] = load_wcol(
                        wq_d if kind == QT_CH else wk_d,
                        dt_i,
                        f"w{'q' if kind == QT_CH else 'k'}{dt_i}",
                    )
                return wcols[key]

            with (
                tc.tile_pool(name="et", bufs=4) as et_pool,
                tc.tile_pool(name="otmp", bufs=2) as otmp_pool,
                tc.tile_pool(name="rr", bufs=2) as rr_pool,
                tc.tile_pool(name="rb", bufs=2) as rb_pool,
                tc.tile_pool(name="ps_s", bufs=2, space="PSUM") as ps_s_pool,
                tc.tile_pool(name="po", bufs=1, space="PSUM") as po_pool,
                tc.tile_pool(name="ps_f", bufs=2, space="PSUM") as ps_f_pool,
            ):
                units = [
                    (p, lh, st)
                    for p in range(NPAIR)
                    for lh in range(2)
                    for st in range(ST)
                ]

                ACT_NS, PE_UNIT_NS = 1030.0, 640.0
                headroom = 0.0
                fill_idx = 0

                def emit_fill(f):
                    _, kind, a, b_, _cost = f
                    if kind == QT_CH:
                        if b_ == 0 and a + 1 < DT:
                            # prefetch next d-tile's weights while this one runs
                            ensure_wcol(QT_CH, a + 1)
                            ensure_wcol(KT_CH, a + 1)
                        qt_chain(ensure_wcol(QT_CH, a), a, b_, ps_f_pool)
                    elif kind == KT_CH:
                        kt_chain(ensure_wcol(KT_CH, a), a, b_, ps_f_pool)
                    else:
                        v_chain(a, b_, ps_f_pool)

                # warm the first fill d-tile's weights up front
                ensure_wcol(QT_CH, 1)
                ensure_wcol(KT_CH, 1)

                def scores(p, lh, st):
                    ps_s = ps_s_pool.tile([P, 2, LH], F32, name="ps_s")
                    for sub in range(2):
                        nc.tensor.matmul(
                            ps_s[:, sub, :],
                            kt[sub * HD : (sub + 1) * HD, p, st * P : (st + 1) * P],
                            qt[sub * HD : (sub + 1) * HD, p, lh * LH : (lh + 1) * LH],
                            start=True,
                            stop=True,
                        )
                    e2 = et_pool.tile([P, 2, LH], BF16, name="et")
                    nc.scalar.activation(e2[:], ps_s[:], AF.Exp, scale=SCALE)
                    return e2

                po_cur = {}

                def pv(p, lh, st, e2):
                    if (p, lh) not in po_cur:
                        po_cur[(p, lh)] = po_pool.tile([HD + 1, 2, LH], F32, name="po")
                    po = po_cur[(p, lh)]
                    for sub in range(2):
                        nc.tensor.matmul(
                            po[:, sub, :],
                            vsb[:, st, 2 * p + sub, 0 : HD + 1],
                            e2[:, sub, :],
                            start=(st == 0),
                            stop=(st == ST - 1),
                        )

                def epilogue(p, lh):
                    po = po_cur.pop((p, lh))
                    o_tmp = otmp_pool.tile([HD + 1, 2, LH], F32, name="o_tmp")
                    nc.vector.tensor_copy(o_tmp[:], po[:])  # frees po banks
                    r_row = rr_pool.tile([1, 2, LH], F32, name="r_row")
                    nc.vector.reciprocal(r_row[:], o_tmp[HD : HD + 1, :, :])
                    r_bc = rb_pool.tile([HD, 2, LH], F32, name="r_bc")
                    nc.gpsimd.partition_broadcast(r_bc[:], r_row[:])
                    for sub in range(2):
                        dst = ot[sub * HD : (sub + 1) * HD, p, lh * LH : (lh + 1) * LH]
                        nc.vector.tensor_mul(dst, o_tmp[0:HD, sub, :], r_bc[:, sub, :])
                        nc.vector.tensor_scalar_add(
                            dst, dst, bv_sb[sub * HD : (sub + 1) * HD, p : p + 1]
                        )

                prev = None
                for ui, u in enumerate(units):
                    # deadline-forced and headroom-allowed fills
                    while fill_idx < len(fills) and (
                        fills[fill_idx][0] <= ui + 4
                        or headroom >= fills[fill_idx][4]
                    ):
                        f = fills[fill_idx]
                        # prefetch next QT/KT weight tiles early
                        emit_fill(f)
                        headroom -= f[4]
                        fill_idx += 1
                    e2 = scores(*u)
                    if prev is not None:
                        pv(*prev[0], prev[1])
                        pp, plh, pst = prev[0]
                        if pst == ST - 1:
                            epilogue(pp, plh)
                    prev = (u, e2)
                    headroom += ACT_NS - PE_UNIT_NS
                for f in fills[fill_idx:]:
                    emit_fill(f)
                pv(*prev[0], prev[1])
                epilogue(prev[0][0], prev[0][1])

        if stop_after == "ab":
            with tc.tile_pool(name="dmp2", bufs=2) as dmp:
                for i in range(DT):
                    t = dmp.tile([P, LQ], F32, name="dmp2_t")
                    nc.vector.tensor_copy(t[:], ot[:, i, :])
                    nc.sync.dma_start(y_d[i * P : (i + 1) * P, :], t[:])
            continue

        # ---------------- output projection: y computed directly ----------
        with (
            tc.tile_pool(name="wo", bufs=2) as wo_pool,
            tc.tile_pool(name="ysl", bufs=3) as y_pool,
            tc.tile_pool(name="ps_y", bufs=3, space="PSUM") as ps_y_pool,
        ):
            for dh in range(2):
                wo_sb = wo_pool.tile([P, DT, LH], BF16, name="wo_sb")
                nc.gpsimd.dma_start(
                    wo_sb[:],
                    wo_d[:, dh * LH : (dh + 1) * LH].rearrange("(t p) n -> p t n", p=P),
                )
                for lt in range(DT):  # 8 l tiles
                    ps_y = ps_y_pool.tile([P, LH], F32, name="ps_y")
                    for ki in range(DT):
                        nc.tensor.matmul(
                            ps_y[:],
                            ot[:, ki, lt * P : (lt + 1) * P],
                            wo_sb[:, ki, :],
                            start=(ki == 0),
                            stop=(ki == DT - 1),
                        )
                    y_sb = y_pool.tile([P, LH], F32, name="y_sb")
                    nc.vector.tensor_add(
                        y_sb[:], ps_y[:], bo_bc[:, dh * LH : (dh + 1) * LH]
                    )
                    nc.sync.dma_start(
                        y_d[lt * P : (lt + 1) * P, dh * LH : (dh + 1) * LH], y_sb[:]
                    )

    nc.finalize()
    return nc


_NC_CACHE = None


def kernel(**inputs):
    global _NC_CACHE
    if _NC_CACHE is None:
        _NC_CACHE = build_nc()
    nc = _NC_CACHE

    q = np.ascontiguousarray(np.asarray(inputs["q"], dtype=np.float32))
    w = {k: np.ascontiguousarray(np.asarray(inputs[k], dtype=np.float32))
         for k in ("Wq", "Wk", "Wv", "Wo", "bq", "bk", "bv", "bo")}

    in_maps = []
    for c in range(N_CORES):
        b, half = c // 2, c % 2
        lo = LQ * half
        x_rot = np.concatenate([q[b, lo:], q[b, :lo]], axis=0)
        in_maps.append({
            "x": np.ascontiguousarray(x_rot),
            "wq": w["Wq"], "wk": w["Wk"], "wv": w["Wv"], "wo": w["Wo"],
            "bq": w["bq"], "bk": w["bk"], "bv": w["bv"], "bo": w["bo"],
        })

    res = run_bass_kernel_spmd(nc, in_maps, core_ids=list(range(N_CORES)))

    out = np.empty((B, L, D), dtype=np.float32)
    for c in range(N_CORES):
        b, half = c // 2, c % 2
        lo = LQ * half
        out[b, lo : lo + LQ, :] = res.results[c]["y"]
    return out


# revision 13
# speedup vs baseline: 1.0414x; 1.0322x over previous
"""Multi-head attention (B=4, L=2048, D=1024, H=16) on 8 NeuronCores.

Sharding: core c handles batch b=c//2 and query rows [1024*(c%2), +1024).
Per-core input x is the batch's [2048, 1024] activations ROTATED so the
core's own query rows are rows 0..1023 (softmax over keys is permutation
invariant). No collectives.

v2 design (vs v1 baseline):
- ACT (ScalarE) does ONLY exp (the hard 263us/core floor at 1 elem/lane/cyc);
  every PSUM eviction moved to DVE (with bias via tensor_scalar_add or a
  broadcast bo row). Single act table load.
- bf16 residents: xt/qt/kt/V/e2/ot (moving-operand bf16 = 1 cyc/row on PE,
  same as f32r at >=256 cols, but halves SBUF so V stays resident in SBUF --
  no DRAM roundtrip for V).
- Scores matmuls (contraction=64) use PE row tiling: sub0 on partitions 0-63
  -> tile (0,0), sub1 on 64-127 -> tile (64,0); the two stream CONCURRENTLY
  (microbenched ~1.8x), halving scores PE time.
- x transposed with a bf16 identity (1 cyc/row vs 2 for fp32) into bf16 xt.
- Output projection computes y[l,:] directly (lhsT=ot l-block, rhs=Wo) --
  no output transpose pass at all.
- Flat (pair, lh, st) unit pipeline with lookahead-1 exp; QT/KT/V projection
  chains for later head-pairs are interleaved into the attention phase as PE
  "fill" work, deadline-scheduled, so PE mops up its surplus while ACT
  streams exp at 100% duty.

PSUM: ps_s (scores) 2 tiles x 2 banks + po (PV accum, [65,2,512], ones-row
denominator trick) 1 tile x 2 banks + fills 2 x 1 bank = 8 banks exactly.
"""

import numpy as np

import sys

for _p in ("/opt/trn_rl_repo", "/opt/pypackages"):
    if _p not in sys.path:
        sys.path.append(_p)

from contextlib import ExitStack

import concourse.bass as bass
import concourse.mybir as mybir
import concourse.tile as tile
from concourse import bacc
from concourse.bass_utils import run_bass_kernel_spmd
from concourse.masks import make_identity

B, L, D, H = 4, 2048, 1024, 16
HD = D // H  # 64
LQ = 1024  # query rows per core
N_CORES = 8
F32 = mybir.dt.float32
F32R = mybir.dt.float32r
BF16 = mybir.dt.bfloat16
AF = mybir.ActivationFunctionType

P = 128
DT = D // P  # 8 d tiles
ST = L // P  # 16 s tiles
LH = 512
NPAIR = H // 2  # 8 head pairs
SCALE = 1.0 / float(np.sqrt(HD))

# fill chain kinds
QT_CH, KT_CH, V_CH = 0, 1, 2


def _load_bias(nc, pool, dram, name):
    """[1024] dram vector -> [128, 8] sbuf tile; column t = b[128t:128t+128]."""
    t = pool.tile([P, DT], F32, name=name)
    nc.gpsimd.dma_start(t[:], dram.rearrange("(t p) -> p t", p=P))
    return t


def build_nc(repeat=1, stop_after=None, scores_serial=False, group2=True):
    nc = bacc.Bacc(None)

    x_d = nc.declare_dram_parameter("x", [L, D], F32, isOutput=False)
    wq_d = nc.declare_dram_parameter("wq", [D, D], F32, isOutput=False)
    wk_d = nc.declare_dram_parameter("wk", [D, D], F32, isOutput=False)
    wv_d = nc.declare_dram_parameter("wv", [D, D], F32, isOutput=False)
    wo_d = nc.declare_dram_parameter("wo", [D, D], F32, isOutput=False)
    bq_d = nc.declare_dram_parameter("bq", [D], F32, isOutput=False)
    bk_d = nc.declare_dram_parameter("bk", [D], F32, isOutput=False)
    bv_d = nc.declare_dram_parameter("bv", [D], F32, isOutput=False)
    bo_d = nc.declare_dram_parameter("bo", [D], F32, isOutput=False)
    y_d = nc.declare_dram_parameter("y", [LQ, D], F32, isOutput=True)

    with tile.TileContext(nc) as tc, ExitStack() as ctx:
      for _rep in range(repeat):
       with ExitStack() as rctx:
        singles = rctx.enter_context(tc.tile_pool(name="singles", bufs=1))
        ident32 = singles.tile([P, P], F32, name="ident32")
        make_identity(nc, ident32[:])
        ident_b = singles.tile([P, P], BF16, name="ident_b")
        nc.vector.tensor_copy(ident_b[:], ident32[:])
        bq_sb = _load_bias(nc, singles, bq_d, "bq")
        bk_sb = _load_bias(nc, singles, bk_d, "bk")
        bv_sb = _load_bias(nc, singles, bv_d, "bv")
        bo_row = singles.tile([1, D], F32, name="bo_row")
        nc.gpsimd.dma_start(bo_row[:], bo_d.rearrange("(a d) -> a d", a=1))
        bo_bc = singles.tile([P, D], F32, name="bo_bc")
        nc.gpsimd.partition_broadcast(bo_bc[:], bo_row[:])

        # big resident slabs (bf16)
        slab = rctx.enter_context(tc.tile_pool(name="slab", bufs=1))
        qt = slab.tile([P, DT, LQ], BF16, name="qt")  # [d%128, dtile, l]
        kt = slab.tile([P, DT, L], BF16, name="kt")  # [d%128, dtile, s]
        ot = slab.tile([P, DT, LQ], BF16, name="ot")  # [din%128, dintile, l]
        # V resident in SBUF: [s%128, st, head, 65]; col 64 = 1.0 (denominator)
        vsb = slab.tile([P, ST, H, HD + 1], BF16, name="vsb")
        nc.vector.memset(vsb[:, :, :, HD : HD + 1], 1.0)

        # wv stays loaded for the V fill chains (rhs moving operand); bf16
        # (gpsimd DMA casts) to fit the SBUF budget
        wv_sb = slab.tile([P, DT, D], BF16, name="wv_sb")
        nc.gpsimd.dma_start(wv_sb[:], wv_d.rearrange("(t p) n -> p t n", p=P))

        with ExitStack() as bctx:
            xt_pool = bctx.enter_context(tc.tile_pool(name="xt", bufs=1))
            xt = xt_pool.tile([P, DT, L], BF16, name="xt")  # [k%128, ktile, s]

            # rotating weight-column tiles for QT/KT fill chains
            wf_pool = bctx.enter_context(tc.tile_pool(name="wf", bufs=4))

            def load_wcol(w_d, dt_i, name):
                w_col = wf_pool.tile([P, DT, P], BF16, name="w_col")
                nc.gpsimd.dma_start(
                    w_col[:],
                    w_d[:, dt_i * P : (dt_i + 1) * P].rearrange(
                        "(t p) n -> p t n", p=P
                    ),
                )
                return w_col

            # ---------------- prefix: transpose + first projections --------
            with (
                tc.tile_pool(name="xpool", bufs=3) as xpool,
                tc.tile_pool(name="ps_tr", bufs=3, space="PSUM") as ps_tr,
                tc.tile_pool(name="ps_pf", bufs=3, space="PSUM") as ps_pf,
            ):
                # transpose x into xt: bf16 identity streams at 1 cyc/row;
                # stationary data bitcast to f32r (bf16-mode truncation is
                # fine -- xt is stored bf16 anyway)
                for li in range(ST):
                    x_sb = xpool.tile([P, D], BF16, name="x_sb")
                    nc.gpsimd.dma_start(x_sb[:], x_d[li * P : (li + 1) * P, :])
                    for kg in range(DT // 4):
                        pt4 = ps_tr.tile([P, 4, P], BF16, name="pt4")
                        for b in range(4):
                            ki = 4 * kg + b
                            nc.tensor.transpose(
                                pt4[:, b, :],
                                x_sb[:, ki * P : (ki + 1) * P],
                                ident_b[:],
                            )
                        nc.vector.tensor_copy(
                            xt[:, 4 * kg : 4 * kg + 4, li * P : (li + 1) * P], pt4[:]
                        )

                def qt_chain(w_col, dt_i, ci, pool):
                    ps = pool.tile([P, LH], F32, name="ps_ch")
                    for ki in range(DT):
                        nc.tensor.matmul(
                            ps[:],
                            w_col[:, ki, :],
                            xt[:, ki, ci * LH : (ci + 1) * LH],
                            start=(ki == 0),
                            stop=(ki == DT - 1),
                        )
                    nc.vector.tensor_scalar_add(
                        qt[:, dt_i, ci * LH : (ci + 1) * LH],
                        ps[:],
                        bq_sb[:, dt_i : dt_i + 1],
                    )

                def kt_chain(w_col, dt_i, ci, pool):
                    ps = pool.tile([P, LH], F32, name="ps_ch")
                    for ki in range(DT):
                        nc.tensor.matmul(
                            ps[:],
                            w_col[:, ki, :],
                            xt[:, ki, ci * LH : (ci + 1) * LH],
                            start=(ki == 0),
                            stop=(ki == DT - 1),
                        )
                    nc.vector.tensor_scalar_add(
                        kt[:, dt_i, ci * LH : (ci + 1) * LH],
                        ps[:],
                        bk_sb[:, dt_i : dt_i + 1],
                    )

                def v_chain(g, st, pool):
                    # V quarter-group g (4 heads = 256 d-cols) for s-tile st
                    ps = pool.tile([P, LH], F32, name="ps_ch")
                    for ki in range(DT):
                        nc.tensor.matmul(
                            ps[:, 0:256],
                            xt[:, ki, st * P : (st + 1) * P],
                            wv_sb[:, ki, g * 256 : (g + 1) * 256],
                            start=(ki == 0),
                            stop=(ki == DT - 1),
                        )
                    # scatter 4 heads' 64-wide blocks into vsb (no bias: bv
                    # is added after normalization in the epilogue)
                    nc.vector.tensor_copy(
                        vsb[:, st, 4 * g : 4 * g + 4, 0:HD],
                        ps[:, 0:256],
                    )

                if stop_after == "tr":
                    with tc.tile_pool(name="dmp0", bufs=2) as dmp:
                        t = dmp.tile([P, LQ], F32, name="dmp0_t")
                        nc.vector.tensor_copy(t[:], xt[:, 0, 0:LQ])
                        nc.sync.dma_start(y_d[0:P, :], t[:])
                    continue

                # prefix chains: QT/KT d-tile 0 and all of V group 0
                wq0 = load_wcol(wq_d, 0, "wq0")
                wk0 = load_wcol(wk_d, 0, "wk0")
                for ci in range(2):
                    qt_chain(wq0, 0, ci, ps_pf)
                for ci in range(4):
                    kt_chain(wk0, 0, ci, ps_pf)
                for st in range(ST):
                    v_chain(0, st, ps_pf)

            if stop_after == "a":
                # dump qt (bf16) widened via DVE into a f32 bounce
                with tc.tile_pool(name="dmp", bufs=2) as dmp:
                    for i in range(DT):
                        t = dmp.tile([P, LQ], F32, name="dmp_t")
                        nc.vector.tensor_copy(t[:], qt[:, i, :])
                        nc.sync.dma_start(y_d[i * P : (i + 1) * P, :], t[:])
                continue

            # ---------------- attention phase with interleaved fills -------
            # fill chains and their deadlines (unit index by which they must
            # be DONE; pair p starts at unit 32p)
            fills = []
            for dt_i in range(1, DT):
                for ci in range(2):
                    fills.append((32 * dt_i, QT_CH, dt_i, ci, 1707))
                for ci in range(4):
                    fills.append((32 * dt_i, KT_CH, dt_i, ci, 1707))
            for g in range(1, 4):
                for st in range(ST):
                    # needed at PV of pair 2g, lh0, st -> unit 64g + st
                    fills.append((64 * g + st, V_CH, g, st, 853))
            fills.sort(key=lambda f: f[0])

            # prefetch weight tiles a couple of d-tiles ahead of use
            wcols = {}

            def ensure_wcol(kind, dt_i):
                key = (kind, dt_i)
                if key not in wcols:
                    wcols[key] = load_wcol(
                        wq_d if kind == QT_CH else wk_d,
                        dt_i,
                        f"w{'q' if kind == QT_CH else 'k'}{dt_i}",
                    )
                return wcols[key]

            with (
                tc.tile_pool(name="et", bufs=4) as et_pool,
                tc.tile_pool(name="otmp", bufs=2) as otmp_pool,
                tc.tile_pool(name="rr", bufs=2) as rr_pool,
                tc.tile_pool(name="rb", bufs=2) as rb_pool,
                tc.tile_pool(name="ps_s", bufs=2, space="PSUM") as ps_s_pool,
                tc.tile_pool(name="po", bufs=1, space="PSUM") as po_pool,
                tc.tile_pool(name="ps_f", bufs=2, space="PSUM") as ps_f_pool,
            ):
                units = [
                    (p, lh, st)
                    for p in range(NPAIR)
                    for lh in range(2)
                    for st in range(ST)
                ]

                ACT_NS, PE_UNIT_NS = 1030.0, 640.0
                headroom = 0.0
                fill_idx = 0

                def emit_fill(f):
                    _, kind, a, b_, _cost = f
                    if kind == QT_CH:
                        if b_ == 0 and a + 1 < DT:
                            # prefetch next d-tile's weights while this one runs
                            ensure_wcol(QT_CH, a + 1)
                            ensure_wcol(KT_CH, a + 1)
                        qt_chain(ensure_wcol(QT_CH, a), a, b_, ps_f_pool)
                    elif kind == KT_CH:
                        kt_chain(ensure_wcol(KT_CH, a), a, b_, ps_f_pool)
                    else:
                        v_chain(a, b_, ps_f_pool)

                # warm the first fill d-tile's weights up front
                ensure_wcol(QT_CH, 1)
                ensure_wcol(KT_CH, 1)

                def scores(p, lh, st):
                    ps_s = ps_s_pool.tile([P, 2, LH], F32, name="ps_s")
                    for sub in range(2):
                        lo = 0 if scores_serial else sub * HD
                        nc.tensor.matmul(
                            ps_s[:, sub, :],
                            kt[lo : lo + HD, p, st * P : (st + 1) * P],
                            qt[lo : lo + HD, p, lh * LH : (lh + 1) * LH],
                            start=True,
                            stop=True,
                        )
                    e2 = et_pool.tile([P, 2, LH], BF16, name="et")
                    nc.scalar.activation(e2[:], ps_s[:], AF.Exp, scale=SCALE)
                    return e2

                po_cur = {}

                def pv(p, lh, st, e2):
                    if (p, lh) not in po_cur:
                        po_cur[(p, lh)] = po_pool.tile([HD + 1, 2, LH], F32, name="po")
                    po = po_cur[(p, lh)]
                    for sub in range(2):
                        nc.tensor.matmul(
                            po[:, sub, :],
                            vsb[:, st, 2 * p + sub, 0 : HD + 1],
                            e2[:, sub, :],
                            start=(st == 0),
                            stop=(st == ST - 1),
                        )

                def epilogue(p, lh):
                    po = po_cur.pop((p, lh))
                    o_tmp = otmp_pool.tile([HD + 1, 2, LH], F32, name="o_tmp")
                    nc.vector.tensor_copy(o_tmp[:], po[:])  # frees po banks
                    r_row = rr_pool.tile([1, 2, LH], F32, name="r_row")
                    nc.vector.reciprocal(r_row[:], o_tmp[HD : HD + 1, :, :])
                    r_bc = rb_pool.tile([HD, 2, LH], F32, name="r_bc")
                    nc.gpsimd.partition_broadcast(r_bc[:], r_row[:])
                    for sub in range(2):
                        dst = ot[sub * HD : (sub + 1) * HD, p, lh * LH : (lh + 1) * LH]
                        nc.vector.tensor_mul(dst, o_tmp[0:HD, sub, :], r_bc[:, sub, :])
                        nc.vector.tensor_scalar_add(
                            dst, dst, bv_sb[sub * HD : (sub + 1) * HD, p : p + 1]
                        )

                def do_fills(ui):
                    nonlocal fill_idx, headroom
                    while fill_idx < len(fills) and (
                        fills[fill_idx][0] <= ui + 4
                        or headroom >= fills[fill_idx][4]
                    ):
                        f = fills[fill_idx]
                        emit_fill(f)
                        headroom -= f[4]
                        fill_idx += 1

                if group2:
                    # units two-at-a-time: S S ... PV PV halves the number of
                    # 64x128 <-> 128x128 PE tile-mode transitions
                    pend = []
                    for gi in range(0, len(units), 2):
                        do_fills(gi)
                        cur = []
                        for u in (units[gi], units[gi + 1]):
                            cur.append((u, scores(*u)))
                        for (pu, pe2) in pend:
                            pv(*pu, pe2)
                            if pu[2] == ST - 1:
                                epilogue(pu[0], pu[1])
                        pend = cur
                        headroom += 2 * (ACT_NS - PE_UNIT_NS)
                    for f in fills[fill_idx:]:
                        emit_fill(f)
                    for (pu, pe2) in pend:
                        pv(*pu, pe2)
                        if pu[2] == ST - 1:
                            epilogue(pu[0], pu[1])
                else:
                    prev = None
                    for ui, u in enumerate(units):
                        do_fills(ui)
                        e2 = scores(*u)
                        if prev is not None:
                            pv(*prev[0], prev[1])
                            pp, plh, pst = prev[0]
                            if pst == ST - 1:
                                epilogue(pp, plh)
                        prev = (u, e2)
                        headroom += ACT_NS - PE_UNIT_NS
                    for f in fills[fill_idx:]:
                        emit_fill(f)
                    pv(*prev[0], prev[1])
                    epilogue(prev[0][0], prev[0][1])

        if stop_after == "ab":
            with tc.tile_pool(name="dmp2", bufs=2) as dmp:
                for i in range(DT):
                    t = dmp.tile([P, LQ], F32, name="dmp2_t")
                    nc.vector.tensor_copy(t[:], ot[:, i, :])
                    nc.sync.dma_start(y_d[i * P : (i + 1) * P, :], t[:])
            continue

        # ---------------- output projection: y computed directly ----------
        with (
            tc.tile_pool(name="wo", bufs=2) as wo_pool,
            tc.tile_pool(name="ysl", bufs=3) as y_pool,
            tc.tile_pool(name="ps_y", bufs=3, space="PSUM") as ps_y_pool,
        ):
            for dh in range(2):
                wo_sb = wo_pool.tile([P, DT, LH], BF16, name="wo_sb")
                nc.gpsimd.dma_start(
                    wo_sb[:],
                    wo_d[:, dh * LH : (dh + 1) * LH].rearrange("(t p) n -> p t n", p=P),
                )
                for lt in range(DT):  # 8 l tiles
                    ps_y = ps_y_pool.tile([P, LH], F32, name="ps_y")
                    for ki in range(DT):
                        nc.tensor.matmul(
                            ps_y[:],
                            ot[:, ki, lt * P : (lt + 1) * P],
                            wo_sb[:, ki, :],
                            start=(ki == 0),
                            stop=(ki == DT - 1),
                        )
                    y_sb = y_pool.tile([P, LH], F32, name="y_sb")
                    nc.vector.tensor_add(
                        y_sb[:], ps_y[:], bo_bc[:, dh * LH : (dh + 1) * LH]
                    )
                    nc.sync.dma_start(
                        y_d[lt * P : (lt + 1) * P, dh * LH : (dh + 1) * LH], y_sb[:]
                    )

    nc.finalize()
    return nc


_NC_CACHE = None


def kernel(**inputs):
    global _NC_CACHE
    if _NC_CACHE is None:
        _NC_CACHE = build_nc()
    nc = _NC_CACHE

    q = np.ascontiguousarray(np.asarray(inputs["q"], dtype=np.float32))
    w = {k: np.ascontiguousarray(np.asarray(inputs[k], dtype=np.float32))
         for k in ("Wq", "Wk", "Wv", "Wo", "bq", "bk", "bv", "bo")}

    in_maps = []
    for c in range(N_CORES):
        b, half = c // 2, c % 2
        lo = LQ * half
        x_rot = np.concatenate([q[b, lo:], q[b, :lo]], axis=0)
        in_maps.append({
            "x": np.ascontiguousarray(x_rot),
            "wq": w["Wq"], "wk": w["Wk"], "wv": w["Wv"], "wo": w["Wo"],
            "bq": w["bq"], "bk": w["bk"], "bv": w["bv"], "bo": w["bo"],
        })

    res = run_bass_kernel_spmd(nc, in_maps, core_ids=list(range(N_CORES)))

    out = np.empty((B, L, D), dtype=np.float32)
    for c in range(N_CORES):
        b, half = c // 2, c % 2
        lo = LQ * half
        out[b, lo : lo + LQ, :] = res.results[c]["y"]
    return out


# revision 15
# speedup vs baseline: 1.5427x; 1.4814x over previous
"""Multi-head attention (B=4, L=2048, D=1024, H=16) on 8 NeuronCores.

Sharding: core c handles batch b=c//2 and query rows [1024*(c%2), +1024).
Each core receives ONLY its own 1024 rows of x. K/V for the other half of
the batch come from the pair partner via pairwise AllGather collectives
(replica groups [2b, 2b+1]) -- softmax over keys is permutation invariant,
and both cores consume keys/values in the same gathered [even|odd] order,
so no reordering is ever needed.

Pipeline per core (all matmuls bf16 in / fp32 psum):
  prefix: DMA-cast x(own) -> bf16, PE-transpose -> xt; QT d-tile 0; KT d-tile
    0 (own cols) -> stage -> AllGather -> load; V group 0 (own s-tiles) ->
    stage (with per-head ones column for the softmax denominator) ->
    AllGather -> load into SBUF-resident vsb.
  B: flat (pair, lh, st) units, two-at-a-time (halves PE tile-mode switches):
    scores = row-tiled matmul pair (contraction 64: sub0 on partitions 0-63
    tile (0,0), sub1 on 64-127 tile (64,0) -- they stream concurrently);
    exp on ACT (the only ACT work in the kernel; ~1 elem/lane/cyc floor);
    PV accumulates [V_h|1]^T @ exp into [65, 2, 512] psum (row 64 =
    denominator). Remaining QT/KT/V projection chains + their collectives
    are interleaved as deadline-scheduled PE fill work so PE mops up its
    surplus while ACT streams exp at full duty.
    Epilogue per (pair, lh): evict, reciprocal of denominator row,
    broadcast, multiply, +bv (bv is added post-normalization: sum(attn)=1).
  C: y computed directly per l-tile (lhsT = ot l-block, rhs = Wo half), +bo
    via a broadcast row; DMA out per (lt, dh) chunk. No output transpose.
"""

import numpy as np

import sys

for _p in ("/opt/trn_rl_repo", "/opt/pypackages"):
    if _p not in sys.path:
        sys.path.append(_p)

from contextlib import ExitStack

import concourse.bass as bass
import concourse.mybir as mybir
import concourse.tile as tile
from concourse import bacc
from concourse.bass_utils import run_bass_kernel_spmd
from concourse.masks import make_identity

B, L, D, H = 4, 2048, 1024, 16
HD = D // H  # 64
LQ = 1024  # query rows per core
N_CORES = 8
F32 = mybir.dt.float32
F32R = mybir.dt.float32r
BF16 = mybir.dt.bfloat16
AF = mybir.ActivationFunctionType

P = 128
DT = D // P  # 8 d tiles
ST = L // P  # 16 s tiles (gathered order)
STO = ST // 2  # 8 own s tiles
LH = 512
NPAIR = H // 2  # 8 head pairs
SCALE = 1.0 / float(np.sqrt(HD))
RG = [[0, 1], [2, 3], [4, 5], [6, 7]]

# fill kinds
QT_CH, KT_CH, V_CH, KT_CC, V_CC = 0, 1, 2, 3, 4


def _load_bias(nc, pool, dram, name):
    t = pool.tile([P, DT], F32, name=name)
    nc.gpsimd.dma_start(t[:], dram.rearrange("(t p) -> p t", p=P))
    return t


def build_nc(repeat=1, stop_after=None, group2=True):
    nc = bacc.Bacc(None)

    x_d = nc.declare_dram_parameter("x", [LQ, D], F32, isOutput=False)
    wq_d = nc.declare_dram_parameter("wq", [D, D], F32, isOutput=False)
    wk_d = nc.declare_dram_parameter("wk", [D, D], F32, isOutput=False)
    wv_d = nc.declare_dram_parameter("wv", [D, D], F32, isOutput=False)
    wo_d = nc.declare_dram_parameter("wo", [D, D], F32, isOutput=False)
    bq_d = nc.declare_dram_parameter("bq", [D], F32, isOutput=False)
    bk_d = nc.declare_dram_parameter("bk", [D], F32, isOutput=False)
    bv_d = nc.declare_dram_parameter("bv", [D], F32, isOutput=False)
    bo_d = nc.declare_dram_parameter("bo", [D], F32, isOutput=False)
    y_d = nc.declare_dram_parameter("y", [LQ, D], F32, isOutput=True)

    with tile.TileContext(nc) as tc, ExitStack() as ctx:
      for _rep in range(repeat):
       with ExitStack() as rctx:
        kt_own_d = [nc.dram_tensor(f"kt_own{_rep}_{i}", [P, LQ], BF16) for i in range(DT)]
        kt_gath_d = [nc.dram_tensor(f"kt_gath{_rep}_{i}", [2, P, LQ], BF16) for i in range(DT)]
        v_own_d = [nc.dram_tensor(f"v_own{_rep}_{g}", [STO, P, 4, HD + 1], BF16) for g in range(4)]
        v_gath_d = [nc.dram_tensor(f"v_gath{_rep}_{g}", [ST, P, 4, HD + 1], BF16) for g in range(4)]

        singles = rctx.enter_context(tc.tile_pool(name="singles", bufs=1))
        ident32 = singles.tile([P, P], F32, name="ident32")
        make_identity(nc, ident32[:])
        ident_b = singles.tile([P, P], BF16, name="ident_b")
        nc.vector.tensor_copy(ident_b[:], ident32[:])
        bq_sb = _load_bias(nc, singles, bq_d, "bq")
        bk_sb = _load_bias(nc, singles, bk_d, "bk")
        bv_sb = _load_bias(nc, singles, bv_d, "bv")
        bo_row = singles.tile([1, D], F32, name="bo_row")
        nc.gpsimd.dma_start(bo_row[:], bo_d.rearrange("(a d) -> a d", a=1))
        bo_bc = singles.tile([P, D], F32, name="bo_bc")
        nc.gpsimd.partition_broadcast(bo_bc[:], bo_row[:])

        slab = rctx.enter_context(tc.tile_pool(name="slab", bufs=1))
        qt = slab.tile([P, DT, LQ], BF16, name="qt")  # [d%128, dtile, l]
        kt = slab.tile([P, DT, L], BF16, name="kt")  # [d%128, dtile, s-gathered]
        ot = slab.tile([P, DT, LQ], BF16, name="ot")  # [din%128, dintile, l]
        vsb = slab.tile([P, ST, H, HD + 1], BF16, name="vsb")  # [s%128, st, h, 65]

        wv_sb = slab.tile([P, DT, D], BF16, name="wv_sb")
        nc.gpsimd.dma_start(wv_sb[:], wv_d.rearrange("(t p) n -> p t n", p=P))

        with ExitStack() as bctx:
            xt_pool = bctx.enter_context(tc.tile_pool(name="xt", bufs=1))
            xt = xt_pool.tile([P, DT, LQ], BF16, name="xt")  # own half only

            wf_pool = bctx.enter_context(tc.tile_pool(name="wf", bufs=4))
            ks_pool = bctx.enter_context(tc.tile_pool(name="ks", bufs=2))
            vb_pool = bctx.enter_context(tc.tile_pool(name="vb", bufs=3))

            def load_wcol(w_d, dt_i):
                w_col = wf_pool.tile([P, DT, P], BF16, name="w_col")
                nc.gpsimd.dma_start(
                    w_col[:],
                    w_d[:, dt_i * P : (dt_i + 1) * P].rearrange(
                        "(t p) n -> p t n", p=P
                    ),
                )
                return w_col

            def qt_chain(w_col, dt_i, ci, pool):
                ps = pool.tile([P, LH], F32, name="ps_ch")
                for ki in range(DT):
                    nc.tensor.matmul(
                        ps[:],
                        w_col[:, ki, :],
                        xt[:, ki, ci * LH : (ci + 1) * LH],
                        start=(ki == 0),
                        stop=(ki == DT - 1),
                    )
                nc.vector.tensor_scalar_add(
                    qt[:, dt_i, ci * LH : (ci + 1) * LH],
                    ps[:],
                    bq_sb[:, dt_i : dt_i + 1],
                )

            def kt_chain(w_col, dt_i, ci, pool):
                # own-half K columns -> staging sbuf (bias included) -> DRAM
                ps = pool.tile([P, LH], F32, name="ps_ch")
                for ki in range(DT):
                    nc.tensor.matmul(
                        ps[:],
                        w_col[:, ki, :],
                        xt[:, ki, ci * LH : (ci + 1) * LH],
                        start=(ki == 0),
                        stop=(ki == DT - 1),
                    )
                kst = ks_pool.tile([P, LH], BF16, name="kst")
                nc.vector.tensor_scalar_add(kst[:], ps[:], bk_sb[:, dt_i : dt_i + 1])
                nc.sync.dma_start(
                    kt_own_d[dt_i][:, ci * LH : (ci + 1) * LH], kst[:]
                )

            def kt_cc(dt_i):
                nc.gpsimd.collective_compute(
                    "AllGather",
                    mybir.AluOpType.bypass,
                    replica_groups=RG,
                    ins=[kt_own_d[dt_i][:]],
                    outs=[kt_gath_d[dt_i][:]],
                )
                nc.sync.dma_start(
                    kt[:, dt_i, :].rearrange("p (a n) -> p a n", a=2),
                    kt_gath_d[dt_i].rearrange("a p n -> p a n"),
                )

            def v_chain(g, st, pool):
                # V quarter-group g (4 heads), own s-tile st -> bounce with
                # ones column -> stage DMA (no bias: bv added post-normalize)
                ps = pool.tile([P, LH], F32, name="ps_ch")
                for ki in range(DT):
                    nc.tensor.matmul(
                        ps[:, 0:256],
                        xt[:, ki, st * P : (st + 1) * P],
                        wv_sb[:, ki, g * 256 : (g + 1) * 256],
                        start=(ki == 0),
                        stop=(ki == DT - 1),
                    )
                vb = vb_pool.tile([P, 4, HD + 1], BF16, name="vb")
                nc.vector.memset(vb[:, :, HD : HD + 1], 1.0)
                nc.vector.tensor_copy(vb[:, :, 0:HD], ps[:, 0:256])
                nc.sync.dma_start(v_own_d[g][st], vb[:])

            def v_cc(g):
                nc.gpsimd.collective_compute(
                    "AllGather",
                    mybir.AluOpType.bypass,
                    replica_groups=RG,
                    ins=[v_own_d[g][:]],
                    outs=[v_gath_d[g][:]],
                )
                nc.sync.dma_start(
                    vsb[:, :, 4 * g : 4 * g + 4, :],
                    v_gath_d[g].rearrange("t p h c -> p t h c"),
                )

            # ---------------- prefix ----------------
            with (
                tc.tile_pool(name="xpool", bufs=3) as xpool,
                tc.tile_pool(name="ps_tr", bufs=3, space="PSUM") as ps_tr,
                tc.tile_pool(name="ps_pf", bufs=3, space="PSUM") as ps_pf,
            ):
                for li in range(STO):
                    x_sb = xpool.tile([P, D], BF16, name="x_sb")
                    nc.gpsimd.dma_start(x_sb[:], x_d[li * P : (li + 1) * P, :])
                    for kg in range(DT // 4):
                        pt4 = ps_tr.tile([P, 4, P], BF16, name="pt4")
                        for b in range(4):
                            ki = 4 * kg + b
                            nc.tensor.transpose(
                                pt4[:, b, :],
                                x_sb[:, ki * P : (ki + 1) * P],
                                ident_b[:],
                            )
                        nc.vector.tensor_copy(
                            xt[:, 4 * kg : 4 * kg + 4, li * P : (li + 1) * P], pt4[:]
                        )

                if stop_after == "tr":
                    with tc.tile_pool(name="dmp0", bufs=2) as dmp:
                        t = dmp.tile([P, LQ], F32, name="dmp0_t")
                        nc.vector.tensor_copy(t[:], xt[:, 0, 0:LQ])
                        nc.sync.dma_start(y_d[0:P, :], t[:])
                    continue

                wq0 = load_wcol(wq_d, 0)
                wk0 = load_wcol(wk_d, 0)
                for ci in range(2):
                    kt_chain(wk0, 0, ci, ps_pf)
                kt_cc(0)
                for st in range(STO):
                    v_chain(0, st, ps_pf)
                v_cc(0)
                for ci in range(2):
                    qt_chain(wq0, 0, ci, ps_pf)

            if stop_after == "a":
                with tc.tile_pool(name="dmp", bufs=2) as dmp:
                    for i in range(DT):
                        t = dmp.tile([P, LQ], F32, name="dmp_t")
                        nc.vector.tensor_copy(t[:], qt[:, i, :])
                        nc.sync.dma_start(y_d[i * P : (i + 1) * P, :], t[:])
                continue

            # ---------------- attention with interleaved fills -------------
            # deadlines in flat-unit index; pair p starts at unit 32p. KT/V
            # fills finish a pair early so their AllGather completes before
            # the consuming pair starts.
            fills = []
            for dt_i in range(1, DT):
                dl = max(0, 32 * (dt_i - 1) - 8)
                for ci in range(2):
                    fills.append((dl, QT_CH, dt_i, ci, 1707))
                for ci in range(2):
                    fills.append((dl, KT_CH, dt_i, ci, 1707))
                fills.append((dl + 1, KT_CC, dt_i, 0, 0))
            for g in range(1, 4):
                dl = max(0, 64 * (g - 1) - 8)
                for st in range(STO):
                    fills.append((dl, V_CH, g, st, 853))
                fills.append((dl + 1, V_CC, g, 0, 0))
            fills.sort(key=lambda f: f[0])

            wcols = {}

            def ensure_wcol(kind, dt_i):
                key = (kind, dt_i)
                if key not in wcols:
                    wcols[key] = load_wcol(wq_d if kind == QT_CH else wk_d, dt_i)
                return wcols[key]

            with (
                tc.tile_pool(name="et", bufs=4) as et_pool,
                tc.tile_pool(name="otmp", bufs=2) as otmp_pool,
                tc.tile_pool(name="rr", bufs=2) as rr_pool,
                tc.tile_pool(name="rb", bufs=2) as rb_pool,
                tc.tile_pool(name="ps_s", bufs=2, space="PSUM") as ps_s_pool,
                tc.tile_pool(name="po", bufs=1, space="PSUM") as po_pool,
                tc.tile_pool(name="ps_f", bufs=2, space="PSUM") as ps_f_pool,
            ):
                units = [
                    (p, lh, st)
                    for p in range(NPAIR)
                    for lh in range(2)
                    for st in range(ST)
                ]

                ACT_NS, PE_UNIT_NS = 1030.0, 640.0
                headroom = 0.0
                fill_idx = 0

                def emit_fill(f):
                    _, kind, a, b_, _cost = f
                    if kind == QT_CH:
                        if b_ == 0 and a + 1 < DT:
                            ensure_wcol(QT_CH, a + 1)
                            ensure_wcol(KT_CH, a + 1)
                        qt_chain(ensure_wcol(QT_CH, a), a, b_, ps_f_pool)
                    elif kind == KT_CH:
                        kt_chain(ensure_wcol(KT_CH, a), a, b_, ps_f_pool)
                    elif kind == V_CH:
                        v_chain(a, b_, ps_f_pool)
                    elif kind == KT_CC:
                        kt_cc(a)
                    else:
                        v_cc(a)

                ensure_wcol(QT_CH, 1)
                ensure_wcol(KT_CH, 1)

                def scores(p, lh, st):
                    ps_s = ps_s_pool.tile([P, 2, LH], F32, name="ps_s")
                    for sub in range(2):
                        nc.tensor.matmul(
                            ps_s[:, sub, :],
                            kt[sub * HD : (sub + 1) * HD, p, st * P : (st + 1) * P],
                            qt[sub * HD : (sub + 1) * HD, p, lh * LH : (lh + 1) * LH],
                            start=True,
                            stop=True,
                        )
                    e2 = et_pool.tile([P, 2, LH], BF16, name="et")
                    nc.scalar.activation(e2[:], ps_s[:], AF.Exp, scale=SCALE)
                    return e2

                po_cur = {}

                def pv(p, lh, st, e2):
                    if (p, lh) not in po_cur:
                        po_cur[(p, lh)] = po_pool.tile([HD + 1, 2, LH], F32, name="po")
                    po = po_cur[(p, lh)]
                    for sub in range(2):
                        nc.tensor.matmul(
                            po[:, sub, :],
                            vsb[:, st, 2 * p + sub, 0 : HD + 1],
                            e2[:, sub, :],
                            start=(st == 0),
                            stop=(st == ST - 1),
                        )

                def epilogue(p, lh):
                    po = po_cur.pop((p, lh))
                    o_tmp = otmp_pool.tile([HD + 1, 2, LH], F32, name="o_tmp")
                    nc.vector.tensor_copy(o_tmp[:], po[:])  # frees po banks
                    r_row = rr_pool.tile([1, 2, LH], F32, name="r_row")
                    nc.vector.reciprocal(r_row[:], o_tmp[HD : HD + 1, :, :])
                    r_bc = rb_pool.tile([HD, 2, LH], F32, name="r_bc")
                    nc.gpsimd.partition_broadcast(r_bc[:], r_row[:])
                    for sub in range(2):
                        dst = ot[sub * HD : (sub + 1) * HD, p, lh * LH : (lh + 1) * LH]
                        nc.vector.tensor_mul(dst, o_tmp[0:HD, sub, :], r_bc[:, sub, :])
                        nc.vector.tensor_scalar_add(
                            dst, dst, bv_sb[sub * HD : (sub + 1) * HD, p : p + 1]
                        )

                def do_fills(ui):
                    nonlocal fill_idx, headroom
                    while fill_idx < len(fills) and (
                        fills[fill_idx][0] <= ui + 4
                        or headroom >= fills[fill_idx][4]
                    ):
                        f = fills[fill_idx]
                        emit_fill(f)
                        headroom -= f[4]
                        fill_idx += 1

                if group2:
                    pend = []
                    for gi in range(0, len(units), 2):
                        do_fills(gi)
                        cur = []
                        for u in (units[gi], units[gi + 1]):
                            cur.append((u, scores(*u)))
                        for (pu, pe2) in pend:
                            pv(*pu, pe2)
                            if pu[2] == ST - 1:
                                epilogue(pu[0], pu[1])
                        pend = cur
                        headroom += 2 * (ACT_NS - PE_UNIT_NS)
                    for f in fills[fill_idx:]:
                        emit_fill(f)
                    for (pu, pe2) in pend:
                        pv(*pu, pe2)
                        if pu[2] == ST - 1:
                            epilogue(pu[0], pu[1])
                else:
                    prev = None
                    for ui, u in enumerate(units):
                        do_fills(ui)
                        e2 = scores(*u)
                        if prev is not None:
                            pv(*prev[0], prev[1])
                            pp, plh, pst = prev[0]
                            if pst == ST - 1:
                                epilogue(pp, plh)
                        prev = (u, e2)
                        headroom += ACT_NS - PE_UNIT_NS
                    for f in fills[fill_idx:]:
                        emit_fill(f)
                    pv(*prev[0], prev[1])
                    epilogue(prev[0][0], prev[0][1])

        if stop_after == "ab":
            with tc.tile_pool(name="dmp2", bufs=2) as dmp:
                for i in range(DT):
                    t = dmp.tile([P, LQ], F32, name="dmp2_t")
                    nc.vector.tensor_copy(t[:], ot[:, i, :])
                    nc.sync.dma_start(y_d[i * P : (i + 1) * P, :], t[:])
            continue

        # ---------------- output projection ----------------
        with (
            tc.tile_pool(name="wo", bufs=2) as wo_pool,
            tc.tile_pool(name="ysl", bufs=3) as y_pool,
            tc.tile_pool(name="ps_y", bufs=3, space="PSUM") as ps_y_pool,
        ):
            for dh in range(2):
                wo_sb = wo_pool.tile([P, DT, LH], BF16, name="wo_sb")
                nc.gpsimd.dma_start(
                    wo_sb[:],
                    wo_d[:, dh * LH : (dh + 1) * LH].rearrange("(t p) n -> p t n", p=P),
                )
                for lt in range(DT):
                    ps_y = ps_y_pool.tile([P, LH], F32, name="ps_y")
                    for ki in range(DT):
                        nc.tensor.matmul(
                            ps_y[:],
                            ot[:, ki, lt * P : (lt + 1) * P],
                            wo_sb[:, ki, :],
                            start=(ki == 0),
                            stop=(ki == DT - 1),
                        )
                    y_sb = y_pool.tile([P, LH], F32, name="y_sb")
                    nc.vector.tensor_add(
                        y_sb[:], ps_y[:], bo_bc[:, dh * LH : (dh + 1) * LH]
                    )
                    nc.sync.dma_start(
                        y_d[lt * P : (lt + 1) * P, dh * LH : (dh + 1) * LH], y_sb[:]
                    )

    nc.finalize()
    return nc


_NC_CACHE = None


def kernel(**inputs):
    global _NC_CACHE
    if _NC_CACHE is None:
        _NC_CACHE = build_nc()
    nc = _NC_CACHE

    q = np.ascontiguousarray(np.asarray(inputs["q"], dtype=np.float32))
    w = {k: np.ascontiguousarray(np.asarray(inputs[k], dtype=np.float32))
         for k in ("Wq", "Wk", "Wv", "Wo", "bq", "bk", "bv", "bo")}

    in_maps = []
    for c in range(N_CORES):
        b, half = c // 2, c % 2
        lo = LQ * half
        in_maps.append({
            "x": np.ascontiguousarray(q[b, lo : lo + LQ]),
            "wq": w["Wq"], "wk": w["Wk"], "wv": w["Wv"], "wo": w["Wo"],
            "bq": w["bq"], "bk": w["bk"], "bv": w["bv"], "bo": w["bo"],
        })

    res = run_bass_kernel_spmd(nc, in_maps, core_ids=list(range(N_CORES)))

    out = np.empty((B, L, D), dtype=np.float32)
    for c in range(N_CORES):
        b, half = c // 2, c % 2
        lo = LQ * half
        out[b, lo : lo + LQ, :] = res.results[c]["y"]
    return out
